# revision 8
# baseline (speedup 1.0000x reference)
"""Trainium2 Bass kernel for nn_MeshNodeBlock (GNN message passing block).

reference semantics:
    agg = segment_sum(edge_features, src_indices, N)        # scatter-add
    x   = concat([node_features, agg], -1)
    h   = silu(x @ W1 + b1)
    y   = h @ W2 + b2
    y   = layer_norm(y) * gamma + beta
    out = y + node_features

Strategy (8 NeuronCores, SPMD, one NEFF):
  * Host snake-deals nodes by degree into 800 bins (8 cores x 100 tiles) of
    128 slots each, so every tile receives ~750 edges = exactly 6 chunks
    of 128 (vs 7 for contiguous partitioning). Edges ship as bf16 features
    (256 B/slot) plus a 2-byte bf16 local id -- no host one-hot (was an
    extra 128 B/slot of pure DMA).
  * Device builds each chunk's [128,128] one-hot on the vector engine in one
    tensor_scalar is_equal against a bf16 iota constant (per-partition lid
    scalar), then scatter-adds with 128-col PE matmuls
    aggT += edge_chunk.T @ onehot into the group's [128,512] PSUM tile.
  * MLP in transposed space (features on partitions): layer 1 -> silu on
    scalar engine (bias fused), layer 2 -> yT. All PSUM->SBUF moves (agg,
    y, y^2) run on the scalar engine (Copy/Square with fused +b2) to keep
    the vector engine free for one-hots.
  * LayerNorm stats via ONCB matmuls (rows of a shared PSUM bank); block
    phase2 computes rstd (ln/exp set) and D = mu*rstd, bounces rows through
    DRAM; phase3 normalizes in 3-4 fused DVE ops
    (y*rstd - D)*gamma + beta + node, interleaved with next block's work.
  * Output written transposed in bf16; host inverts the node permutation.
"""

import functools
from contextlib import ExitStack

import numpy as np
import ml_dtypes

import concourse.bass as bass
import concourse.tile as tile
from concourse import bacc, mybir
from concourse import bass_utils

BF16 = ml_dtypes.bfloat16

N_NODES = 100000
D = 128
N_CORES = 8
P = 128
GROUP = 512
N_GROUPS = 25
TILES_PER_CORE = 100
NODES_PER_CORE = N_GROUPS * GROUP   # 12800
NBINS = N_CORES * TILES_PER_CORE    # 800
EPS = 1e-5

AF = mybir.ActivationFunctionType
ALU = mybir.AluOpType
dt = mybir.dt


# --------------------------------------------------------------------------
# device kernel builder
# --------------------------------------------------------------------------

@functools.lru_cache(maxsize=4)
def _build(cis: tuple, n_cores: int, beta_zero: bool):
    assert len(cis) == TILES_PER_CORE
    coff = np.concatenate([[0], np.cumsum(cis)]).astype(int)
    # per-group chunk counts / byte offsets (features then lids per group)
    gch = [int(coff[4 * g + 4] - coff[4 * g]) for g in range(N_GROUPS)]
    gbytes = [c * 260 for c in gch]   # 256 B bf16 features + 4 B f32 lid
    boff = np.concatenate([[0], np.cumsum(gbytes)]).astype(int)
    gbmax = max(gbytes)

    ntail = 4
    blocks = [list(range(0, N_GROUPS - ntail)),
              list(range(N_GROUPS - ntail, N_GROUPS))]
    bmax = max(len(b) for b in blocks)

    nc = bacc.Bacc("TRN2", target_bir_lowering=False, debug=False,
                   enable_asserts=False, num_devices=n_cores)

    PK = nc.dram_tensor("pk", [P, int(boff[-1])], dt.uint8,
                        kind="ExternalInput").ap()
    NTB = nc.dram_tensor("ntb", [P, NODES_PER_CORE], dt.bfloat16,
                         kind="ExternalInput").ap()
    W1P = nc.dram_tensor("w1p", [P, 1024], dt.bfloat16, kind="ExternalInput").ap()
    W2P = nc.dram_tensor("w2p", [P, 512], dt.bfloat16, kind="ExternalInput").ap()
    B1P = nc.dram_tensor("b1p", [P, 4], dt.float32, kind="ExternalInput").ap()
    B2P = nc.dram_tensor("b2p", [P, 1], dt.float32, kind="ExternalInput").ap()
    GAM = nc.dram_tensor("gam", [P, 1], dt.float32, kind="ExternalInput").ap()
    BET = nc.dram_tensor("bet", [P, 1], dt.float32, kind="ExternalInput").ap()
    ONB = nc.dram_tensor("onb", [P, bmax * 128], dt.bfloat16,
                         kind="ExternalInput").ap()
    IOT = nc.dram_tensor("iot", [P, P], dt.bfloat16,
                         kind="ExternalInput").ap()
    OUT = nc.dram_tensor("out", [P, NODES_PER_CORE], dt.bfloat16,
                         kind="ExternalOutput").ap()

    with tile.TileContext(nc) as tc:
        with ExitStack() as ctx:
            singles = ctx.enter_context(tc.tile_pool(name="singles", bufs=1))
            pkp = ctx.enter_context(tc.tile_pool(name="pkp", bufs=4))
            ohp = ctx.enter_context(tc.tile_pool(name="ohp", bufs=56))
            xtp = ctx.enter_context(tc.tile_pool(name="xtp", bufs=N_GROUPS + 2))
            xap = ctx.enter_context(tc.tile_pool(name="xap", bufs=3))
            shp = ctx.enter_context(tc.tile_pool(name="shp", bufs=6))
            yp = ctx.enter_context(tc.tile_pool(name="yp", bufs=N_GROUPS + 2))
            y2p = ctx.enter_context(tc.tile_pool(name="y2p", bufs=bmax + 2))
            stp = ctx.enter_context(tc.tile_pool(name="stp", bufs=2))
            zp = ctx.enter_context(tc.tile_pool(name="zp", bufs=3))
            psagg = ctx.enter_context(tc.tile_pool(name="psagg", bufs=2, space="PSUM"))
            psh = ctx.enter_context(tc.tile_pool(name="psh", bufs=2, space="PSUM"))
            psy = ctx.enter_context(tc.tile_pool(name="psy", bufs=2, space="PSUM"))
            psst = ctx.enter_context(tc.tile_pool(name="psst", bufs=1, space="PSUM"))
            drp = ctx.enter_context(tc.tile_pool(name="drp", bufs=2, space="DRAM"))

            def load_const(name, src, shape, dtyp):
                t = singles.tile(shape, dtyp, tag=name)
                nc.sync.dma_start(out=t[:], in_=src)
                return t

            w1 = load_const("w1", W1P, [P, 1024], dt.bfloat16)
            w2 = load_const("w2", W2P, [P, 512], dt.bfloat16)
            b1 = load_const("b1", B1P, [P, 4], dt.float32)
            b2 = load_const("b2", B2P, [P, 1], dt.float32)
            gam = load_const("gam", GAM, [P, 1], dt.float32)
            bet = load_const("bet", BET, [P, 1], dt.float32)
            onb = load_const("onb", ONB, [P, bmax * 128], dt.bfloat16)
            iot = load_const("iot", IOT, [P, P], dt.bfloat16)

            pk_tiles = {}
            xtn_tiles = {}
            oh_tiles = {}
            y_tiles = {}
            y2_tiles = {}

            def dma_group(g):
                if g >= N_GROUPS:
                    return
                pkt = pkp.tile([P, gbmax], dt.uint8, tag="pk")
                nc.sync.dma_start(out=pkt[:, :gbytes[g]],
                                  in_=PK[:, int(boff[g]):int(boff[g]) + gbytes[g]])
                pk_tiles[g] = pkt
                xtn = xtp.tile([P, GROUP], dt.bfloat16, tag="xtn")
                nc.sync.dma_start(out=xtn[:],
                                  in_=NTB[:, g * GROUP:(g + 1) * GROUP])
                xtn_tiles[g] = xtn

            def onehots(g):
                if g >= N_GROUPS:
                    return
                pkt = pk_tiles[g]
                nch = gch[g]
                lid0 = nch * 256
                for lc in range(nch):
                    lid = pkt[:, lid0 + lc * 4:lid0 + (lc + 1) * 4].bitcast(
                        dt.float32)
                    oh = ohp.tile([P, P], dt.bfloat16, tag="oh")
                    nc.vector.tensor_scalar(out=oh[:], in0=iot[:],
                                            scalar1=lid, scalar2=None,
                                            op0=ALU.is_equal)
                    oh_tiles[(g, lc)] = oh

            def scatter_mlp(g):
                agg_ps = psagg.tile([P, GROUP], dt.float32, tag="agg")
                pkt = pk_tiles.pop(g)
                a = int(coff[4 * g])
                for t4 in range(4):
                    ti = 4 * g + t4
                    ci = int(cis[ti])
                    for c in range(ci):
                        lc = int(coff[ti]) + c - a
                        ebv = pkt[:, lc * 256:(lc + 1) * 256].bitcast(
                            dt.bfloat16)
                        nc.tensor.matmul(
                            out=agg_ps[:, t4 * 128:(t4 + 1) * 128],
                            lhsT=ebv, rhs=oh_tiles.pop((g, lc))[:],
                            start=(c == 0), stop=(c == ci - 1))
                xta = xap.tile([P, GROUP], dt.bfloat16, tag="xta")
                nc.scalar.activation(out=xta[:], in_=agg_ps[:], func=AF.Copy)
                xtn = xtn_tiles[g]
                sh_tiles = []
                for j in range(4):
                    hps = psh.tile([P, GROUP], dt.float32, tag="hps")
                    nc.tensor.matmul(out=hps[:],
                                     lhsT=w1[:, j * 128:(j + 1) * 128],
                                     rhs=xtn[:], start=True, stop=False)
                    nc.tensor.matmul(
                        out=hps[:],
                        lhsT=w1[:, 512 + j * 128:512 + (j + 1) * 128],
                        rhs=xta[:], start=False, stop=True)
                    sh = shp.tile([P, GROUP], dt.bfloat16, tag=f"sh{j}")
                    nc.scalar.activation(out=sh[:], in_=hps[:], func=AF.Silu,
                                         bias=b1[:, j:j + 1], scale=1.0)
                    sh_tiles.append(sh)
                yps = psy.tile([P, GROUP], dt.float32, tag="yps")
                for j in range(4):
                    nc.tensor.matmul(out=yps[:],
                                     lhsT=w2[:, j * 128:(j + 1) * 128],
                                     rhs=sh_tiles[j][:],
                                     start=(j == 0), stop=(j == 3))
                y = yp.tile([P, GROUP], dt.bfloat16, tag="y")
                nc.scalar.activation(out=y[:], in_=yps[:], func=AF.Identity,
                                     bias=b2[:, 0:1], scale=1.0)
                y_tiles[g] = y
                y2 = y2p.tile([P, GROUP], dt.bfloat16, tag="y2")
                nc.scalar.activation(out=y2[:], in_=yps[:], func=AF.Square,
                                     bias=b2[:, 0:1], scale=1.0)
                y2_tiles[g] = y2

            def stats_burst(block):
                bsz = len(block)
                mu_ps = psst.tile([P, GROUP], dt.float32, tag="mups")
                m2_ps = psst.tile([P, GROUP], dt.float32, tag="m2ps")
                for gi, g in enumerate(block):
                    onc_g = onb[:, gi * 128:(gi + 1) * 128]
                    nc.tensor.matmul(out=mu_ps[:], lhsT=onc_g,
                                     rhs=y_tiles[g][:],
                                     start=(gi == 0), stop=(gi == bsz - 1),
                                     skip_group_check=True)
                    nc.tensor.matmul(out=m2_ps[:], lhsT=onc_g,
                                     rhs=y2_tiles.pop(g)[:],
                                     start=(gi == 0), stop=(gi == bsz - 1),
                                     skip_group_check=True)
                return mu_ps, m2_ps

            def phase2(block, mu_ps, m2_ps):
                bsz = len(block)
                mu_bf = stp.tile([P, GROUP], dt.bfloat16, tag="mubf")
                nc.scalar.activation(out=mu_bf[:], in_=mu_ps[:], func=AF.Copy)
                musq = stp.tile([P, GROUP], dt.bfloat16, tag="musq")
                nc.scalar.activation(out=musq[:], in_=mu_ps[:], func=AF.Square)
                m2_bf = stp.tile([P, GROUP], dt.bfloat16, tag="m2bf")
                nc.scalar.activation(out=m2_bf[:], in_=m2_ps[:], func=AF.Copy)
                # var + eps = (m2 + eps) - mu^2, one fused DVE op
                var = stp.tile([P, GROUP], dt.bfloat16, tag="var")
                nc.vector.scalar_tensor_tensor(
                    out=var[:], in0=m2_bf[:], scalar=EPS, in1=musq[:],
                    op0=ALU.add, op1=ALU.subtract)
                lnv = stp.tile([P, GROUP], dt.bfloat16, tag="lnv")
                nc.scalar.activation(out=lnv[:], in_=var[:], func=AF.Ln)
                rstd = stp.tile([P, GROUP], dt.bfloat16, tag="rstd")
                nc.scalar.activation(out=rstd[:], in_=lnv[:], func=AF.Exp,
                                     bias=0.0, scale=-0.5)
                dmu = stp.tile([P, GROUP], dt.bfloat16, tag="dmu")
                nc.vector.tensor_tensor(out=dmu[:], in0=mu_bf[:],
                                        in1=rstd[:], op=ALU.mult)
                bounce = drp.tile([bsz, 1024], dt.bfloat16, tag="bounce")
                nc.gpsimd.dma_start(out=bounce[:, 0:512], in_=dmu[0:bsz, :])
                nc.gpsimd.dma_start(out=bounce[:, 512:1024],
                                    in_=rstd[0:bsz, :])
                return bounce

            def phase3(g, gi, bounce):
                nsl = slice(g * GROUP, (g + 1) * GROUP)
                mr = zp.tile([P, 1024], dt.bfloat16, tag="mr")
                bsl = bounce[gi:gi + 1, 0:1024]
                nc.gpsimd.dma_start(out=mr[:], in_=bass.AP(
                    tensor=bsl.tensor, offset=bsl.offset,
                    ap=[[0, P], bsl.ap[1]]))
                y = y_tiles.pop(g)
                xtn = xtn_tiles.pop(g)
                t1 = zp.tile([P, GROUP], dt.bfloat16, tag="t1")
                nc.vector.tensor_tensor(out=t1[:], in0=y[:],
                                        in1=mr[:, 512:1024], op=ALU.mult)
                t2 = zp.tile([P, GROUP], dt.bfloat16, tag="t2")
                nc.vector.scalar_tensor_tensor(
                    out=t2[:], in0=mr[:, 0:512], scalar=-1.0, in1=t1[:],
                    op0=ALU.mult, op1=ALU.add)
                of = zp.tile([P, GROUP], dt.bfloat16, tag="of")
                if beta_zero:
                    nc.vector.scalar_tensor_tensor(
                        out=of[:], in0=t2[:], scalar=gam[:, 0:1], in1=xtn[:],
                        op0=ALU.mult, op1=ALU.add)
                else:
                    t3 = zp.tile([P, GROUP], dt.bfloat16, tag="t3")
                    nc.vector.tensor_scalar(out=t3[:], in0=t2[:],
                                            scalar1=gam[:, 0:1],
                                            scalar2=bet[:, 0:1],
                                            op0=ALU.mult, op1=ALU.add)
                    nc.vector.tensor_tensor(out=of[:], in0=t3[:], in1=xtn[:],
                                            op=ALU.add)
                nc.gpsimd.dma_start(out=OUT[:, nsl], in_=of[:])

            # ---- emission ----
            LOOK = 2
            dma_group(0)
            dma_group(1)
            onehots(0)
            prev = None   # (block, bounce) pending phase3
            for bi, block in enumerate(blocks):
                p3queue = list(prev[0]) if prev else []
                for g in block:
                    dma_group(g + LOOK)
                    onehots(g + 1)
                    scatter_mlp(g)
                    # interleave previous block's normalize so DVE/PE overlap
                    if p3queue:
                        g2 = p3queue.pop(0)
                        phase3(g2, prev[0].index(g2), prev[1])
                mu_ps, m2_ps = stats_burst(block)
                if prev:
                    for g2 in p3queue:
                        phase3(g2, prev[0].index(g2), prev[1])
                bounce = phase2(block, mu_ps, m2_ps)
                prev = (block, bounce)
            for g2 in prev[0]:
                phase3(g2, prev[0].index(g2), prev[1])

    nc.compile()
    return nc


# --------------------------------------------------------------------------
# host-side sharding / packing
# --------------------------------------------------------------------------

def _preprocess(inputs):
    nf = np.ascontiguousarray(np.asarray(inputs["node_features"], np.float32))
    ef = np.ascontiguousarray(np.asarray(inputs["edge_features"], np.float32))
    src = np.asarray(inputs["src_indices"]).astype(np.int64)
    W1 = np.asarray(inputs["W1"], np.float32)
    b1 = np.asarray(inputs["b1"], np.float32)
    W2 = np.asarray(inputs["W2"], np.float32)
    b2 = np.asarray(inputs["b2"], np.float32)
    gam = np.asarray(inputs["ln_gamma"], np.float32)
    bet = np.asarray(inputs["ln_beta"], np.float32)

    n_nodes, d = nf.shape
    n_edges = ef.shape[0]
    assert n_nodes == N_NODES and d == D

    # degree-balanced snake deal of nodes into 800 bins of 128 slots
    deg = np.bincount(src, minlength=n_nodes)
    order = np.argsort(-deg, kind="stable")
    idx = np.arange(n_nodes)
    r = idx // NBINS
    c = idx % NBINS
    b = np.where(r % 2 == 0, c, NBINS - 1 - c)
    bin_of = np.empty(n_nodes, np.int64)
    slot_of = np.empty(n_nodes, np.int64)
    bin_of[order] = b
    slot_of[order] = r
    assert slot_of.max() < P

    bindeg = np.bincount(bin_of, weights=deg, minlength=NBINS).astype(np.int64)
    cis = np.ceil(bindeg.reshape(N_CORES, TILES_PER_CORE) / P).astype(int)
    cis = np.maximum(cis.max(axis=0), 1)
    coff = np.concatenate([[0], np.cumsum(cis)]).astype(int)
    chtot = int(coff[-1])

    # edge placement: sort by (bin, lid), chunk within bin
    eb = bin_of[src]
    lid = slot_of[src]
    eorder = np.argsort(eb * P + lid, kind="stable")
    sb = eb[eorder]
    counts = np.bincount(eb, minlength=NBINS)
    starts = np.concatenate([[0], np.cumsum(counts)[:-1]])
    rank = np.arange(n_edges, dtype=np.int64) - starts[sb]
    chunk = rank // P
    pslot = rank % P
    core_e = sb // TILES_PER_CORE
    ti_e = sb % TILES_PER_CORE
    gc = coff[ti_e] + chunk

    EB = np.zeros((N_CORES, P, chtot, D), BF16)
    EB[core_e, pslot, gc, :] = ef[eorder].astype(BF16)
    LID = np.zeros((N_CORES, P, chtot), np.float32)
    LID[core_e, pslot, gc] = lid[eorder].astype(np.float32)

    EB8 = EB.view(np.uint8).reshape(N_CORES, P, chtot * 256)
    L8 = LID.view(np.uint8).reshape(N_CORES, P, chtot * 4)
    parts = []
    for g in range(N_GROUPS):
        a, e = int(coff[4 * g]), int(coff[4 * g + 4])
        parts.append(EB8[:, :, a * 256:e * 256])
        parts.append(L8[:, :, a * 4:e * 4])
    PKa = np.ascontiguousarray(np.concatenate(parts, axis=2))

    # permuted node features, transposed
    core_n = bin_of // TILES_PER_CORE
    col_n = (bin_of % TILES_PER_CORE) * P + slot_of
    NT = np.zeros((N_CORES, NODES_PER_CORE, D), np.float32)
    NT[core_n, col_n] = nf
    NTBa = np.ascontiguousarray(NT.transpose(0, 2, 1)).astype(BF16)

    W1P = np.ascontiguousarray(
        W1.reshape(2, P, 4, P).transpose(1, 0, 2, 3).reshape(P, 1024)).astype(BF16)
    W2P = np.ascontiguousarray(
        W2.reshape(4, P, P).transpose(1, 0, 2).reshape(P, 512)).astype(BF16)
    B1P = np.ascontiguousarray(b1.reshape(4, P).T)
    B2P = np.ascontiguousarray(b2.reshape(P, 1))
    GAMP = np.ascontiguousarray(gam.reshape(P, 1))
    BETP = np.ascontiguousarray(bet.reshape(P, 1))
    bmax = N_GROUPS - 4
    ONBa = np.zeros((P, bmax * 128), np.float32)
    for g in range(bmax):
        ONBa[:, g * 128 + g] = 1.0 / P
    ONBa = ONBa.astype(BF16)
    IOTa = np.ascontiguousarray(
        np.tile(np.arange(P, dtype=np.float32)[None, :], (P, 1))).astype(BF16)

    in_maps = []
    for k in range(N_CORES):
        in_maps.append({
            "pk": PKa[k], "ntb": NTBa[k],
            "w1p": W1P, "w2p": W2P, "b1p": B1P, "b2p": B2P,
            "gam": GAMP, "bet": BETP, "onb": ONBa, "iot": IOTa,
        })
    meta = (core_n, col_n)
    return in_maps, tuple(int(x) for x in cis), bool(np.all(bet == 0.0)), meta


def _assemble(results, meta):
    core_n, col_n = meta
    outs = np.stack([np.asarray(r["out"]) for r in results]).astype(np.float32)
    full = outs[core_n, :, col_n]          # [n_nodes, D]
    return np.ascontiguousarray(full)


# --------------------------------------------------------------------------
# public entry point
# --------------------------------------------------------------------------

_AXON_SO = "/opt/axon/libaxon_pjrt.so"


def _ensure_ntff_hook():
    """Provide antenv.axon_hooks + register the ctypes NTFF profile hook
    (the agent image's antenv lacks axon_hooks, so boot degraded silently)."""
    import sys
    import types
    import ctypes
    import contextlib
    import os

    try:
        from antenv.axon_hooks import get_axon_ntff_profile_hook  # noqa: F401
        return
    except ImportError:
        pass
    import antenv

    m = types.ModuleType("antenv.axon_hooks")
    m._hook = None

    def set_axon_ntff_profile_hook(h):
        m._hook = h

    def get_axon_ntff_profile_hook():
        return m._hook

    m.set_axon_ntff_profile_hook = set_axon_ntff_profile_hook
    m.get_axon_ntff_profile_hook = get_axon_ntff_profile_hook
    sys.modules["antenv.axon_hooks"] = m
    antenv.axon_hooks = m

    if not os.path.exists(_AXON_SO):
        return
    lib = ctypes.CDLL(_AXON_SO)
    if not hasattr(lib, "axon_start_nrt_profile"):
        return
    lib.axon_start_nrt_profile.argtypes = [ctypes.POINTER(ctypes.c_int64),
                                           ctypes.c_size_t]
    lib.axon_start_nrt_profile.restype = ctypes.c_int64
    lib.axon_stop_nrt_profile.argtypes = [ctypes.c_char_p]
    lib.axon_stop_nrt_profile.restype = ctypes.c_int64

    @contextlib.contextmanager
    def _hook(output_dir, device_ids):
        import jax

        jax.devices()
        if device_ids:
            ids = (ctypes.c_int64 * len(device_ids))(*device_ids)
            rc = lib.axon_start_nrt_profile(ids, len(device_ids))
        else:
            rc = lib.axon_start_nrt_profile(None, 0)
        if rc != 0:
            raise RuntimeError(f"axon_start_nrt_profile rc={rc}")
        try:
            yield
        finally:
            n = lib.axon_stop_nrt_profile(str(output_dir).encode())
            if n < 0:
                raise RuntimeError(f"axon_stop_nrt_profile rc={n}")
            if n == 0:
                print("WARNING: NTFF capture wrote no files")

    m._hook = _hook


def _run(inputs, trace=False):
    if trace:
        _ensure_ntff_hook()
    in_maps, cis, beta_zero, meta = _preprocess(inputs)
    nc = _build(cis, N_CORES, beta_zero)
    res = bass_utils.run_bass_kernel_spmd(
        nc, in_maps, core_ids=list(range(N_CORES)), trace=trace)
    out = _assemble(res.results, meta)
    return out, res


def kernel(**inputs):
    out, _ = _run(inputs, trace=False)
    return out


def kernel_profiled(**inputs):
    out, res = _run(inputs, trace=True)
    return out, res


# revision 9
# speedup vs baseline: 1.0690x; 1.0690x over previous
"""Trainium2 Bass kernel for nn_MeshNodeBlock (GNN message passing block).

reference semantics:
    agg = segment_sum(edge_features, src_indices, N)        # scatter-add
    x   = concat([node_features, agg], -1)
    h   = silu(x @ W1 + b1)
    y   = h @ W2 + b2
    y   = layer_norm(y) * gamma + beta
    out = y + node_features

Strategy (8 NeuronCores, SPMD, one NEFF):
  * Host snake-deals nodes by degree into 800 bins (8 cores x 100 tiles) of
    128 slots each, so every tile receives ~750 edges = exactly 6 chunks of
    128 (a contiguous partition needs 7). Each chunk ships bf16 edge
    features (256 B/slot) + fp8 one-hot (128 B/slot).
  * Device works fully in transposed space (features on partitions, nodes on
    free dim). Per 128-node tile the scatter-add is ci PE matmuls
    aggT += edge_chunk.T @ onehot into the group's [128,512] PSUM tile.
  * MLP consumes aggT/nodeT directly: layer 1 -> silu(+b1) on the scalar
    engine, layer 2 -> yT. xta copy on scalar engine; y (+b2) and y^2 on
    the vector engine (y^2 from SBUF, 2x mode).
  * LayerNorm stats via ONCB matmuls (rows of a shared PSUM bank); block
    phase2 computes rstd (ln/exp set) and D = mu*rstd, writes [D|rstd] rows
    to a DRAM bounce; phase3 prefetches them partition-broadcast in one
    large DMA per 7-group sub-block (not 128 tiny replicated reads per
    group) and normalizes in 3-4 fused DVE ops
    (y*rstd - D)*gamma + beta + node, interleaved with next block's work.
  * Output written transposed in bf16; host inverts the node permutation.
"""

import functools
from contextlib import ExitStack

import numpy as np
import ml_dtypes

import concourse.bass as bass
import concourse.tile as tile
from concourse import bacc, mybir
from concourse import bass_utils

BF16 = ml_dtypes.bfloat16
FP8 = ml_dtypes.float8_e4m3

N_NODES = 100000
D = 128
N_CORES = 8
P = 128
GROUP = 512
N_GROUPS = 25
TILES_PER_CORE = 100
NODES_PER_CORE = N_GROUPS * GROUP   # 12800
NBINS = N_CORES * TILES_PER_CORE    # 800
SUB = 7                             # groups per phase3 broadcast prefetch
EPS = 1e-5

AF = mybir.ActivationFunctionType
ALU = mybir.AluOpType
dt = mybir.dt


# --------------------------------------------------------------------------
# device kernel builder
# --------------------------------------------------------------------------

@functools.lru_cache(maxsize=4)
def _build(cis: tuple, n_cores: int, beta_zero: bool):
    assert len(cis) == TILES_PER_CORE
    coff = np.concatenate([[0], np.cumsum(cis)]).astype(int)
    # group chunk counts and pk byte offsets (384 B per chunk-column)
    gch = [int(coff[4 * g + 4] - coff[4 * g]) for g in range(N_GROUPS)]
    gbytes = [c * 384 for c in gch]
    boff = np.concatenate([[0], np.cumsum(gbytes)]).astype(int)
    gbmax = max(gbytes)

    ntail = 4
    blocks = [list(range(0, N_GROUPS - ntail)),
              list(range(N_GROUPS - ntail, N_GROUPS))]
    bmax = max(len(b) for b in blocks)

    nc = bacc.Bacc("TRN2", target_bir_lowering=False, debug=False,
                   enable_asserts=False, num_devices=n_cores)

    PK = nc.dram_tensor("pk", [P, int(boff[-1])], dt.uint8,
                        kind="ExternalInput").ap()
    NTB = nc.dram_tensor("ntb", [P, NODES_PER_CORE], dt.bfloat16,
                         kind="ExternalInput").ap()
    W1P = nc.dram_tensor("w1p", [P, 1024], dt.bfloat16, kind="ExternalInput").ap()
    W2P = nc.dram_tensor("w2p", [P, 512], dt.bfloat16, kind="ExternalInput").ap()
    B1P = nc.dram_tensor("b1p", [P, 4], dt.float32, kind="ExternalInput").ap()
    B2P = nc.dram_tensor("b2p", [P, 1], dt.float32, kind="ExternalInput").ap()
    GAM = nc.dram_tensor("gam", [P, 1], dt.float32, kind="ExternalInput").ap()
    BET = nc.dram_tensor("bet", [P, 1], dt.float32, kind="ExternalInput").ap()
    ONB = nc.dram_tensor("onb", [P, bmax * 128], dt.bfloat16,
                         kind="ExternalInput").ap()
    OUT = nc.dram_tensor("out", [P, NODES_PER_CORE], dt.bfloat16,
                         kind="ExternalOutput").ap()

    with tile.TileContext(nc) as tc:
        with ExitStack() as ctx:
            singles = ctx.enter_context(tc.tile_pool(name="singles", bufs=1))
            pkp = ctx.enter_context(tc.tile_pool(name="pkp", bufs=3))
            xtp = ctx.enter_context(tc.tile_pool(name="xtp", bufs=N_GROUPS + 2))
            xap = ctx.enter_context(tc.tile_pool(name="xap", bufs=3))
            shp = ctx.enter_context(tc.tile_pool(name="shp", bufs=6))
            yp = ctx.enter_context(tc.tile_pool(name="yp", bufs=N_GROUPS + 2))
            y2p = ctx.enter_context(tc.tile_pool(name="y2p", bufs=bmax + 2))
            stp = ctx.enter_context(tc.tile_pool(name="stp", bufs=1))
            mrp = ctx.enter_context(tc.tile_pool(name="mrp", bufs=2))
            zp = ctx.enter_context(tc.tile_pool(name="zp", bufs=3))
            psagg = ctx.enter_context(tc.tile_pool(name="psagg", bufs=2, space="PSUM"))
            psh = ctx.enter_context(tc.tile_pool(name="psh", bufs=2, space="PSUM"))
            psy = ctx.enter_context(tc.tile_pool(name="psy", bufs=2, space="PSUM"))
            psst = ctx.enter_context(tc.tile_pool(name="psst", bufs=1, space="PSUM"))
            drp = ctx.enter_context(tc.tile_pool(name="drp", bufs=2, space="DRAM"))

            def load_const(name, src, shape, dtyp):
                t = singles.tile(shape, dtyp, tag=name)
                nc.sync.dma_start(out=t[:], in_=src)
                return t

            w1 = load_const("w1", W1P, [P, 1024], dt.bfloat16)
            w2 = load_const("w2", W2P, [P, 512], dt.bfloat16)
            b1 = load_const("b1", B1P, [P, 4], dt.float32)
            b2 = load_const("b2", B2P, [P, 1], dt.float32)
            gam = load_const("gam", GAM, [P, 1], dt.float32)
            bet = load_const("bet", BET, [P, 1], dt.float32)
            onb = load_const("onb", ONB, [P, bmax * 128], dt.bfloat16)

            pk_tiles = {}
            xtn_tiles = {}
            y_tiles = {}
            y2_tiles = {}

            def dma_group(g):
                if g >= N_GROUPS:
                    return
                pkt = pkp.tile([P, gbmax], dt.uint8, tag="pk")
                nc.sync.dma_start(out=pkt[:, :gbytes[g]],
                                  in_=PK[:, int(boff[g]):int(boff[g]) + gbytes[g]])
                pk_tiles[g] = pkt
                xtn = xtp.tile([P, GROUP], dt.bfloat16, tag="xtn")
                nc.sync.dma_start(out=xtn[:],
                                  in_=NTB[:, g * GROUP:(g + 1) * GROUP])
                xtn_tiles[g] = xtn

            def scatter_mlp(g):
                agg_ps = psagg.tile([P, GROUP], dt.float32, tag="agg")
                pkt = pk_tiles.pop(g)
                a = int(coff[4 * g])
                for t4 in range(4):
                    ti = 4 * g + t4
                    ci = int(cis[ti])
                    toff = (int(coff[ti]) - a) * 384
                    ebv = pkt[:, toff:toff + ci * 256].bitcast(dt.bfloat16)
                    ohv = pkt[:, toff + ci * 256:toff + ci * 384].bitcast(
                        dt.float8e4)
                    for c in range(ci):
                        nc.tensor.matmul(
                            out=agg_ps[:, t4 * 128:(t4 + 1) * 128],
                            lhsT=ebv[:, c * 128:(c + 1) * 128],
                            rhs=ohv[:, c * 128:(c + 1) * 128],
                            start=(c == 0), stop=(c == ci - 1))
                xta = xap.tile([P, GROUP], dt.bfloat16, tag="xta")
                nc.scalar.activation(out=xta[:], in_=agg_ps[:], func=AF.Copy)
                xtn = xtn_tiles[g]
                sh_tiles = []
                for j in range(4):
                    hps = psh.tile([P, GROUP], dt.float32, tag="hps")
                    nc.tensor.matmul(out=hps[:],
                                     lhsT=w1[:, j * 128:(j + 1) * 128],
                                     rhs=xtn[:], start=True, stop=False)
                    nc.tensor.matmul(
                        out=hps[:],
                        lhsT=w1[:, 512 + j * 128:512 + (j + 1) * 128],
                        rhs=xta[:], start=False, stop=True)
                    sh = shp.tile([P, GROUP], dt.bfloat16, tag=f"sh{j}")
                    nc.scalar.activation(out=sh[:], in_=hps[:], func=AF.Silu,
                                         bias=b1[:, j:j + 1], scale=1.0)
                    sh_tiles.append(sh)
                yps = psy.tile([P, GROUP], dt.float32, tag="yps")
                for j in range(4):
                    nc.tensor.matmul(out=yps[:],
                                     lhsT=w2[:, j * 128:(j + 1) * 128],
                                     rhs=sh_tiles[j][:],
                                     start=(j == 0), stop=(j == 3))
                y = yp.tile([P, GROUP], dt.bfloat16, tag="y")
                nc.vector.tensor_scalar(out=y[:], in0=yps[:],
                                        scalar1=b2[:, 0:1], scalar2=None,
                                        op0=ALU.add)
                y_tiles[g] = y
                y2 = y2p.tile([P, GROUP], dt.bfloat16, tag="y2")
                nc.vector.tensor_tensor(out=y2[:], in0=y[:], in1=y[:],
                                        op=ALU.mult)
                y2_tiles[g] = y2

            def stats_burst(block):
                bsz = len(block)
                mu_ps = psst.tile([P, GROUP], dt.float32, tag="mups")
                m2_ps = psst.tile([P, GROUP], dt.float32, tag="m2ps")
                for gi, g in enumerate(block):
                    onc_g = onb[:, gi * 128:(gi + 1) * 128]
                    nc.tensor.matmul(out=mu_ps[:], lhsT=onc_g,
                                     rhs=y_tiles[g][:],
                                     start=(gi == 0), stop=(gi == bsz - 1),
                                     skip_group_check=True)
                    nc.tensor.matmul(out=m2_ps[:], lhsT=onc_g,
                                     rhs=y2_tiles.pop(g)[:],
                                     start=(gi == 0), stop=(gi == bsz - 1),
                                     skip_group_check=True)
                return mu_ps, m2_ps

            def phase2(block, mu_ps, m2_ps):
                bsz = len(block)
                mu_bf = stp.tile([P, GROUP], dt.bfloat16, tag="mubf")
                nc.scalar.activation(out=mu_bf[:], in_=mu_ps[:], func=AF.Copy)
                musq = stp.tile([P, GROUP], dt.bfloat16, tag="musq")
                nc.scalar.activation(out=musq[:], in_=mu_ps[:], func=AF.Square)
                m2_bf = stp.tile([P, GROUP], dt.bfloat16, tag="m2bf")
                nc.scalar.activation(out=m2_bf[:], in_=m2_ps[:], func=AF.Copy)
                # var + eps = (m2 + eps) - mu^2, one fused DVE op
                var = stp.tile([P, GROUP], dt.bfloat16, tag="var")
                nc.vector.scalar_tensor_tensor(
                    out=var[:], in0=m2_bf[:], scalar=EPS, in1=musq[:],
                    op0=ALU.add, op1=ALU.subtract)
                lnv = stp.tile([P, GROUP], dt.bfloat16, tag="lnv")
                nc.scalar.activation(out=lnv[:], in_=var[:], func=AF.Ln)
                rstd = stp.tile([P, GROUP], dt.bfloat16, tag="rstd")
                nc.scalar.activation(out=rstd[:], in_=lnv[:], func=AF.Exp,
                                     bias=0.0, scale=-0.5)
                dmu = stp.tile([P, GROUP], dt.bfloat16, tag="dmu")
                nc.vector.tensor_tensor(out=dmu[:], in0=mu_bf[:],
                                        in1=rstd[:], op=ALU.mult)
                # [D | rstd] rows, flattened so sub-block prefetches are
                # contiguous per group: row g holds [dmu_g, rstd_g].
                bounce = drp.tile([bsz, 1024], dt.bfloat16, tag="bounce")
                nc.gpsimd.dma_start(out=bounce[:, 0:512], in_=dmu[0:bsz, :])
                nc.gpsimd.dma_start(out=bounce[:, 512:1024],
                                    in_=rstd[0:bsz, :])
                return bounce

            mr_cur = [None, 0]   # (tile, base group-index within block)

            def prefetch_mr(bounce, gi0, cnt):
                mra = mrp.tile([P, SUB * 1024], dt.bfloat16, tag="mra")
                bsl = bounce[gi0:gi0 + cnt, 0:1024]
                nc.gpsimd.dma_start(
                    out=mra[:, :cnt * 1024],
                    in_=bass.AP(tensor=bsl.tensor, offset=bsl.offset,
                                ap=[[0, P], [1, cnt * 1024]]))
                mr_cur[0] = mra
                mr_cur[1] = gi0

            def phase3(g, gi, block, bounce):
                if gi % SUB == 0:
                    prefetch_mr(bounce, gi, min(SUB, len(block) - gi))
                mra = mr_cur[0]
                k = gi - mr_cur[1]
                mr_d = mra[:, k * 1024:k * 1024 + 512]
                mr_r = mra[:, k * 1024 + 512:(k + 1) * 1024]
                nsl = slice(g * GROUP, (g + 1) * GROUP)
                y = y_tiles.pop(g)
                xtn = xtn_tiles.pop(g)
                t1 = zp.tile([P, GROUP], dt.bfloat16, tag="t1")
                nc.vector.tensor_tensor(out=t1[:], in0=y[:], in1=mr_r,
                                        op=ALU.mult)
                t2 = zp.tile([P, GROUP], dt.bfloat16, tag="t2")
                nc.vector.scalar_tensor_tensor(
                    out=t2[:], in0=mr_d, scalar=-1.0, in1=t1[:],
                    op0=ALU.mult, op1=ALU.add)
                of = zp.tile([P, GROUP], dt.bfloat16, tag="of")
                if beta_zero:
                    nc.vector.scalar_tensor_tensor(
                        out=of[:], in0=t2[:], scalar=gam[:, 0:1], in1=xtn[:],
                        op0=ALU.mult, op1=ALU.add)
                else:
                    t3 = zp.tile([P, GROUP], dt.bfloat16, tag="t3")
                    nc.vector.tensor_scalar(out=t3[:], in0=t2[:],
                                            scalar1=gam[:, 0:1],
                                            scalar2=bet[:, 0:1],
                                            op0=ALU.mult, op1=ALU.add)
                    nc.vector.tensor_tensor(out=of[:], in0=t3[:], in1=xtn[:],
                                            op=ALU.add)
                nc.gpsimd.dma_start(out=OUT[:, nsl], in_=of[:])

            # ---- emission ----
            LOOK = 2
            dma_group(0)
            dma_group(1)
            prev = None   # (block, bounce) pending phase3
            for bi, block in enumerate(blocks):
                p3queue = list(prev[0]) if prev else []
                for g in block:
                    dma_group(g + LOOK)
                    scatter_mlp(g)
                    # interleave previous block's normalize
                    if p3queue:
                        g2 = p3queue.pop(0)
                        phase3(g2, prev[0].index(g2), prev[0], prev[1])
                mu_ps, m2_ps = stats_burst(block)
                if prev:
                    for g2 in p3queue:
                        phase3(g2, prev[0].index(g2), prev[0], prev[1])
                bounce = phase2(block, mu_ps, m2_ps)
                prev = (block, bounce)
            for g2 in prev[0]:
                phase3(g2, prev[0].index(g2), prev[0], prev[1])

    nc.compile()
    return nc


# --------------------------------------------------------------------------
# host-side sharding / packing
# --------------------------------------------------------------------------

def _preprocess(inputs):
    nf = np.ascontiguousarray(np.asarray(inputs["node_features"], np.float32))
    ef = np.ascontiguousarray(np.asarray(inputs["edge_features"], np.float32))
    src = np.asarray(inputs["src_indices"]).astype(np.int64)
    W1 = np.asarray(inputs["W1"], np.float32)
    b1 = np.asarray(inputs["b1"], np.float32)
    W2 = np.asarray(inputs["W2"], np.float32)
    b2 = np.asarray(inputs["b2"], np.float32)
    gam = np.asarray(inputs["ln_gamma"], np.float32)
    bet = np.asarray(inputs["ln_beta"], np.float32)

    n_nodes, d = nf.shape
    n_edges = ef.shape[0]
    assert n_nodes == N_NODES and d == D

    # degree-balanced snake deal of nodes into 800 bins of 128 slots
    deg = np.bincount(src, minlength=n_nodes)
    order = np.argsort(-deg, kind="stable")
    idx = np.arange(n_nodes)
    r = idx // NBINS
    c = idx % NBINS
    b = np.where(r % 2 == 0, c, NBINS - 1 - c)
    bin_of = np.empty(n_nodes, np.int64)
    slot_of = np.empty(n_nodes, np.int64)
    bin_of[order] = b
    slot_of[order] = r
    assert slot_of.max() < P

    bindeg = np.bincount(bin_of, weights=deg, minlength=NBINS).astype(np.int64)
    cis = np.ceil(bindeg.reshape(N_CORES, TILES_PER_CORE) / P).astype(int)
    cis = np.maximum(cis.max(axis=0), 1)
    coff = np.concatenate([[0], np.cumsum(cis)]).astype(int)
    chtot = int(coff[-1])

    # edge placement: sort by (bin, lid), chunk within bin
    eb = bin_of[src]
    lid = slot_of[src]
    eorder = np.argsort(eb * P + lid, kind="stable")
    sb = eb[eorder]
    counts = np.bincount(eb, minlength=NBINS)
    starts = np.concatenate([[0], np.cumsum(counts)[:-1]])
    rank = np.arange(n_edges, dtype=np.int64) - starts[sb]
    chunk = rank // P
    pslot = rank % P
    core_e = sb // TILES_PER_CORE
    ti_e = sb % TILES_PER_CORE
    gc = coff[ti_e] + chunk

    EB = np.zeros((N_CORES, P, chtot, D), BF16)
    EB[core_e, pslot, gc, :] = ef[eorder].astype(BF16)
    OH = np.zeros((N_CORES, P, chtot, P), FP8)
    OH[core_e, pslot, gc, lid[eorder]] = 1.0

    EB8 = EB.view(np.uint8).reshape(N_CORES, P, chtot * 256)
    OH8 = OH.view(np.uint8).reshape(N_CORES, P, chtot * 128)
    parts = []
    for ti in range(TILES_PER_CORE):
        a, e = int(coff[ti]), int(coff[ti + 1])
        parts.append(EB8[:, :, a * 256:e * 256])
        parts.append(OH8[:, :, a * 128:e * 128])
    PKa = np.ascontiguousarray(np.concatenate(parts, axis=2))

    # permuted node features, transposed
    core_n = bin_of // TILES_PER_CORE
    col_n = (bin_of % TILES_PER_CORE) * P + slot_of
    NT = np.zeros((N_CORES, NODES_PER_CORE, D), np.float32)
    NT[core_n, col_n] = nf
    NTBa = np.ascontiguousarray(NT.transpose(0, 2, 1)).astype(BF16)

    W1P = np.ascontiguousarray(
        W1.reshape(2, P, 4, P).transpose(1, 0, 2, 3).reshape(P, 1024)).astype(BF16)
    W2P = np.ascontiguousarray(
        W2.reshape(4, P, P).transpose(1, 0, 2).reshape(P, 512)).astype(BF16)
    B1P = np.ascontiguousarray(b1.reshape(4, P).T)
    B2P = np.ascontiguousarray(b2.reshape(P, 1))
    GAMP = np.ascontiguousarray(gam.reshape(P, 1))
    BETP = np.ascontiguousarray(bet.reshape(P, 1))
    bmax = N_GROUPS - 4
    ONBa = np.zeros((P, bmax * 128), np.float32)
    for g in range(bmax):
        ONBa[:, g * 128 + g] = 1.0 / P
    ONBa = ONBa.astype(BF16)

    in_maps = []
    for k in range(N_CORES):
        in_maps.append({
            "pk": PKa[k], "ntb": NTBa[k],
            "w1p": W1P, "w2p": W2P, "b1p": B1P, "b2p": B2P,
            "gam": GAMP, "bet": BETP, "onb": ONBa,
        })
    meta = (core_n, col_n)
    return in_maps, tuple(int(x) for x in cis), bool(np.all(bet == 0.0)), meta


def _assemble(results, meta):
    core_n, col_n = meta
    outs = np.stack([np.asarray(r["out"]) for r in results]).astype(np.float32)
    full = outs[core_n, :, col_n]          # [n_nodes, D]
    return np.ascontiguousarray(full)


# --------------------------------------------------------------------------
# public entry point
# --------------------------------------------------------------------------

_AXON_SO = "/opt/axon/libaxon_pjrt.so"


def _ensure_ntff_hook():
    """Provide antenv.axon_hooks + register the ctypes NTFF profile hook
    (the agent image's antenv lacks axon_hooks, so boot degraded silently)."""
    import sys
    import types
    import ctypes
    import contextlib
    import os

    try:
        from antenv.axon_hooks import get_axon_ntff_profile_hook  # noqa: F401
        return
    except ImportError:
        pass
    import antenv

    m = types.ModuleType("antenv.axon_hooks")
    m._hook = None

    def set_axon_ntff_profile_hook(h):
        m._hook = h

    def get_axon_ntff_profile_hook():
        return m._hook

    m.set_axon_ntff_profile_hook = set_axon_ntff_profile_hook
    m.get_axon_ntff_profile_hook = get_axon_ntff_profile_hook
    sys.modules["antenv.axon_hooks"] = m
    antenv.axon_hooks = m

    if not os.path.exists(_AXON_SO):
        return
    lib = ctypes.CDLL(_AXON_SO)
    if not hasattr(lib, "axon_start_nrt_profile"):
        return
    lib.axon_start_nrt_profile.argtypes = [ctypes.POINTER(ctypes.c_int64),
                                           ctypes.c_size_t]
    lib.axon_start_nrt_profile.restype = ctypes.c_int64
    lib.axon_stop_nrt_profile.argtypes = [ctypes.c_char_p]
    lib.axon_stop_nrt_profile.restype = ctypes.c_int64

    @contextlib.contextmanager
    def _hook(output_dir, device_ids):
        import jax

        jax.devices()
        if device_ids:
            ids = (ctypes.c_int64 * len(device_ids))(*device_ids)
            rc = lib.axon_start_nrt_profile(ids, len(device_ids))
        else:
            rc = lib.axon_start_nrt_profile(None, 0)
        if rc != 0:
            raise RuntimeError(f"axon_start_nrt_profile rc={rc}")
        try:
            yield
        finally:
            n = lib.axon_stop_nrt_profile(str(output_dir).encode())
            if n < 0:
                raise RuntimeError(f"axon_stop_nrt_profile rc={n}")
            if n == 0:
                print("WARNING: NTFF capture wrote no files")

    m._hook = _hook


def _run(inputs, trace=False):
    if trace:
        _ensure_ntff_hook()
    in_maps, cis, beta_zero, meta = _preprocess(inputs)
    nc = _build(cis, N_CORES, beta_zero)
    res = bass_utils.run_bass_kernel_spmd(
        nc, in_maps, core_ids=list(range(N_CORES)), trace=trace)
    out = _assemble(res.results, meta)
    return out, res


def kernel(**inputs):
    out, _ = _run(inputs, trace=False)
    return out


def kernel_profiled(**inputs):
    out, res = _run(inputs, trace=True)
    return out, res
